# revision 25
# baseline (speedup 1.0000x reference)
"""GPT-2 style attention block (B=2, S=2048, D=1024, H=16) on 8 TRN2 NeuronCores.

Sharding: tensor-parallel over heads + data-parallel over batch.
Cores 0-3 handle batch 0, cores 4-7 handle batch 1; each core owns 4 of the
16 heads (its 256-column slice of the qkv projection and the matching
256-row slice of c_proj_w). Each core produces a partial output
[S, D] (stored fp16) = ctx_heads @ c_proj_rows; the 4 partials per batch
are summed on the host.

v2 design (fp16 dataflow, measured v1 trace drove these choices):
  1. hs fp32 DMA -> ACT cast to fp16 -> PE transposes (fp16 1-pass, batched
     4 per PSUM bank) -> DVE cast -> hsT fp16.
  2. QKV projections all-fp16 operands (fp32 PSUM accum): 1024-col moving
     chunks; bias applied via ACT Identity drain (fp16 out) -> qkT.
  3. Attention per (qb, hp, kt) with exact causal trimming: diagonal tiles
     stream only the valid 512-128j query columns; a single [128,128]
     upper-triangular fp16 mask handles the in-tile triangle via DVE mul.
     exp on ACT (both heads of the pair in one op, fp32 psum -> fp16 es).
     AV accumulates ctx_aug^T[65, q] in PSUM (ones-column of V_aug gives
     the softmax denominator in row 64).
  4. Normalize: reciprocal_approx_fast on the denominator rows (PSUM ->
     SBUF), gpsimd partition_broadcast, DVE mul -> ctxT fp16.
  5. Output projection fp16 (1024-col moving), DVE PSUM->fp16 copy, DMA out.
PSUM: tp pool (2x1 bank, closed early) / big pool 2x[128,1024] (projections,
scores, outproj) / cx pool 2x[65,2,512]; 8 banks total.

The bias rows (c_attn_b v-slice folded through c_proj_w, plus c_proj_b)
are added on the host during unsharding (exactly zero for the reference
setup_inputs). The causal_mask input is the deterministic tril mask from
setup_inputs(); causality is implemented analytically on device.
"""

import numpy as np

B, S, D, H = 2, 2048, 1024, 16
HD = D // H  # 64
N_CORES = 8
HPC = 4  # heads per core
GROUPS = 4  # cores per batch
HSL = HPC * HD  # 256: per-core head-column width

NDT = D // 128  # 8 contraction tiles
NRT = S // 128  # 16 row tiles
NQB = S // 512  # 4 query blocks

_nc_cache = {}


def _build():
    import concourse.bacc as bacc
    import concourse.mybir as mybir
    import concourse.tile as tile
    from concourse.masks import make_identity, make_upper_triangular

    f32 = mybir.dt.float32
    f16 = mybir.dt.float16

    nc = bacc.Bacc("TRN2", debug=False, num_devices=N_CORES)

    hs = nc.dram_tensor("hs", [S, D], f32, kind="ExternalInput")
    wqk = nc.dram_tensor("wqk", [D, 3 * HSL], f16, kind="ExternalInput")
    wp = nc.dram_tensor("wp", [HSL, D], f16, kind="ExternalInput")
    bqk = nc.dram_tensor("bqk", [2 * HSL], f32, kind="ExternalInput")
    outp = nc.dram_tensor("outp", [S, D], f16, kind="ExternalOutput")

    with tile.TileContext(nc) as tc:
        with (
            tc.tile_pool(name="persist", bufs=1) as persist,
            tc.tile_pool(name="hs_in", bufs=3) as hs_pool,
            tc.tile_pool(name="hs16", bufs=3) as h16_pool,
            tc.tile_pool(name="es", bufs=6) as es_pool,
            tc.tile_pool(name="rb", bufs=2) as rb_pool,
            tc.tile_pool(name="ob", bufs=3) as ob_pool,
            tc.tile_pool(name="big", bufs=2, space="PSUM") as big_pool,
        ):
            # ---- persistent SBUF ----
            # rt-major so the PSUM->SBUF cast after each row-tile's
            # transposes writes a contiguous 1024-element span
            hsT = persist.tile([128, NRT, NDT, 128], f16)
            qkT = persist.tile([128, 6, S], f16)  # [Q(2) | K(2) | V^T(2)]
            ones64 = persist.tile([1, 64], f16)
            vv = persist.tile([128, NRT, HPC * (HD + 1)], f16)  # V aug
            wqk_sb = persist.tile([128, NDT, 3 * HSL], f16)
            wp_sb = persist.tile([128, 2, D], f16)
            bqk_sb = persist.tile([128, 4], f32)
            ident = persist.tile([128, 128], f16)
            dmask = persist.tile([128, 128], f16)  # 1 where q(col) >= k(row)
            ctxT = persist.tile([128, 2, S], f16)

            make_identity(nc, ident)
            make_upper_triangular(nc, dmask, val=1.0, diag=True)
            # ones columns of V_aug (data cols overwritten by vproj)
            nc.gpsimd.memset(vv, 1.0)
            nc.vector.memset(ones64, 1.0)

            # ---- phases 1-3, software pipelined per 1024-col s-block:
            # DMA+cast+transpose row tiles, V projection per row tile,
            # then the QK^T projection for the completed s-block.
            tp_ctx = tc.tile_pool(name="tp", bufs=2, space="PSUM")
            tp_pool = tp_ctx.__enter__()
            for ntb in range(S // 1024):
                for rt in range(8 * ntb, 8 * ntb + 8):
                    h_in = hs_pool.tile([128, D], f32)
                    nc.sync.dma_start(
                        out=h_in, in_=hs[rt * 128 : (rt + 1) * 128, :]
                    )
                    h16 = h16_pool.tile([128, D], f16)
                    nc.scalar.copy(h16, h_in)
                    pt = tp_pool.tile([128, 1024], f16, tag="tp")
                    for dt in range(NDT):
                        nc.tensor.transpose(
                            pt[:, dt * 128 : (dt + 1) * 128],
                            h16[:, dt * 128 : (dt + 1) * 128],
                            ident,
                        )
                    nc.vector.tensor_copy(
                        hsT[:, rt, :, :],
                        pt.rearrange("p (t c) -> p t c", c=128),
                    )
                    if rt == 0:
                        # weight DMAs after the first hs chunk is queued
                        nc.sync.dma_start(
                            out=wqk_sb,
                            in_=wqk.rearrange("(t p) n -> p t n", p=128),
                        )
                        nc.sync.dma_start(
                            out=wp_sb,
                            in_=wp.rearrange("(t p) n -> p t n", p=128),
                        )
                        nc.sync.dma_start(
                            out=bqk_sb, in_=bqk.rearrange("(t p) -> p t", p=128)
                        )
                # QKV^T projection for this s-block (W stationary)
                for ct in range(6):
                    pj = big_pool.tile([128, 1024], f32, tag="pj")
                    for dt in range(NDT):
                        for half in range(2):
                            nc.tensor.matmul(
                                pj[:, half * 512 : (half + 1) * 512],
                                wqk_sb[:, dt, ct * 128 : (ct + 1) * 128],
                                hsT[
                                    :,
                                    8 * ntb + 4 * half : 8 * ntb + 4 * half + 4,
                                    dt,
                                    :,
                                ],
                                start=(dt == 0),
                                stop=(dt == NDT - 1),
                            )
                    if ct < 4:
                        nc.scalar.activation(
                            qkT[:, ct, ntb * 1024 : (ntb + 1) * 1024],
                            pj,
                            mybir.ActivationFunctionType.Identity,
                            bias=bqk_sb[:, ct : ct + 1],
                        )
                    else:
                        nc.scalar.copy(
                            qkT[:, ct, ntb * 1024 : (ntb + 1) * 1024], pj
                        )
                # V natural layout: PE re-transpose of V^T
                for rt in range(8 * ntb, 8 * ntb + 8):
                    pt2 = tp_pool.tile([128, 256], f16, tag="vt")
                    for vt in range(2):
                        nc.tensor.transpose(
                            pt2[:, vt * 128 : (vt + 1) * 128],
                            qkT[:, 4 + vt, rt * 128 : (rt + 1) * 128],
                            ident,
                        )
                    vtgt = vv[:, rt, :].rearrange("p (h c) -> p h c", c=HD + 1)
                    nc.vector.tensor_copy(
                        vtgt[:, :, 0:HD],
                        pt2.rearrange("p (h c) -> p h c", c=HD),
                    )
            tp_ctx.__exit__(None, None, None)

            # ---- phase 4: attention + output projection ----
            # All normalize/outproj work dependent on a block's softmax
            # denominator is EMITTED one block late: the PE queue is FIFO,
            # so ready attention matmuls stay ahead of chain-dependent ops
            # (denominator broadcast outer-product, outproj) while the
            # reciprocal completes in the background on DVE.
            def emit_outproj(qb, act_ob):
                for mt in range(4 * qb, 4 * qb + 4):
                    po = big_pool.tile([128, 1024], f32, tag="pj")
                    for ht in range(2):
                        for half in range(2):
                            nc.tensor.matmul(
                                po[:, half * 512 : (half + 1) * 512],
                                ctxT[:, ht, mt * 128 : (mt + 1) * 128],
                                wp_sb[:, ht, half * 512 : (half + 1) * 512],
                                start=(ht == 0),
                                stop=(ht == 1),
                            )
                    ob = ob_pool.tile([128, 1024], f16)
                    if act_ob:
                        nc.scalar.copy(ob, po)
                    else:
                        nc.vector.tensor_copy(ob, po)
                    nc.sync.dma_start(
                        out=outp[mt * 128 : (mt + 1) * 128, :], in_=ob
                    )

            def flush_norm(p, last):
                qb, hp, cx, rec3 = p
                for hh in range(2):
                    # broadcast 1/denom across partitions 64-127 of the cx
                    # psum tile via a rank-1 outer product on the PE
                    nc.tensor.matmul(
                        cx[64:128, hh, :],
                        ones64,
                        rec3[:, hh, :],
                        start=True,
                        stop=True,
                    )
                rbt = rb_pool.tile([64, 1024], f16, tag="rbt")
                rbt3 = rbt.rearrange("p (h c) -> p h c", c=512)
                nc.vector.tensor_copy(rbt3, cx[64:128, :, :])
                for hh in range(2):
                    nc.vector.tensor_mul(
                        ctxT[
                            hh * 64 : hh * 64 + 64,
                            hp,
                            qb * 512 : (qb + 1) * 512,
                        ],
                        cx[0:64, hh, :],
                        rbt3[:, hh, :],
                    )
                if hp == 1:
                    emit_outproj(qb, act_ob=last)

            cx_ctx = tc.tile_pool(name="cx", bufs=2, space="PSUM")
            cx_pool = cx_ctx.__enter__()
            pend = None
            for qb, hp in ((0, 0), (0, 1), (3, 0), (3, 1),
                           (2, 0), (2, 1), (1, 0), (1, 1)):
                    cx = cx_pool.tile([128, 2, 512], f32, tag="cx")
                    kmax = 4 * (qb + 1)
                    for kt in range(kmax):
                        j = kt - 4 * qb
                        w = 512 if j < 0 else 512 - 128 * j
                        qo = 512 - w
                        scp = big_pool.tile([128, 1024], f32, tag="pj")
                        for hh in range(2):
                            nc.tensor.matmul(
                                scp[:, 512 * hh + qo : 512 * (hh + 1)],
                                qkT[
                                    hh * 64 : (hh + 1) * 64,
                                    2 + hp,
                                    kt * 128 : (kt + 1) * 128,
                                ],
                                qkT[
                                    hh * 64 : (hh + 1) * 64,
                                    hp,
                                    qb * 512 + qo : (qb + 1) * 512,
                                ],
                                start=True,
                                stop=True,
                                tile_position=(hh * 64, 0),
                            )
                        es = es_pool.tile([128, 1024], f16, tag="es")
                        scp3 = scp.rearrange("p (h c) -> p h c", c=512)
                        es3 = es.rearrange("p (h c) -> p h c", c=512)
                        nc.scalar.activation(
                            es3[:, :, qo:512],
                            scp3[:, :, qo:512],
                            mybir.ActivationFunctionType.Exp,
                            scale=float(1.0 / np.sqrt(HD)),
                        )
                        if j >= 0:
                            for hh in range(2):
                                nc.vector.tensor_mul(
                                    es[:, 512 * hh + qo : 512 * hh + qo + 128],
                                    es[:, 512 * hh + qo : 512 * hh + qo + 128],
                                    dmask,
                                )
                        for hh in range(2):
                            h = 2 * hp + hh
                            nc.tensor.matmul(
                                cx[0:65, hh, qo:512],
                                vv[:, kt, h * (HD + 1) : (h + 1) * (HD + 1)],
                                es[:, 512 * hh + qo : 512 * (hh + 1)],
                                start=(kt == 0),
                                stop=(kt == kmax - 1),
                            )
                    # flush the previous block's chain, then start this
                    # block's reciprocals (row 64 = softmax denominator)
                    if pend is not None:
                        flush_norm(pend, last=False)
                    rec = rb_pool.tile([1, 1024], f16, tag="rec")
                    rec3 = rec.rearrange("p (h c) -> p h c", c=512)
                    for hh in range(2):
                        with nc.allow_low_precision("softmax denom recip"):
                            nc.vector.reciprocal(
                                rec3[:, hh, :], cx[64:65, hh, :]
                            )
                    pend = (qb, hp, cx, rec3)
            flush_norm(pend, last=True)
            cx_ctx.__exit__(None, None, None)

    nc.compile()
    return nc


def build_kernel(matmul_dtype=None, av_dtype=None):
    # single fp16 variant; dtype args accepted for harness compat
    if "k" not in _nc_cache:
        _nc_cache["k"] = _build()
    return _nc_cache["k"]


def make_in_maps(
    hidden_states, c_attn_w, c_attn_b, c_proj_w, c_proj_b,
    matmul_dtype=None, av_dtype=None,
):
    hidden_states = np.asarray(hidden_states, dtype=np.float32)
    c_attn_w = np.asarray(c_attn_w, dtype=np.float32)
    c_attn_b = np.asarray(c_attn_b, dtype=np.float32)
    c_proj_w = np.asarray(c_proj_w, dtype=np.float32)
    c_proj_b = np.asarray(c_proj_b, dtype=np.float32)

    in_maps = []
    for c in range(N_CORES):
        b, g = divmod(c, GROUPS)
        cs = slice(g * HSL, (g + 1) * HSL)
        wq = c_attn_w[:, g * HSL : (g + 1) * HSL]
        wk = c_attn_w[:, D + g * HSL : D + (g + 1) * HSL]
        wvs = c_attn_w[:, 2 * D + g * HSL : 2 * D + (g + 1) * HSL]
        bq = c_attn_b[g * HSL : (g + 1) * HSL]
        bk = c_attn_b[D + g * HSL : D + (g + 1) * HSL]
        bv = c_attn_b[2 * D + g * HSL : 2 * D + (g + 1) * HSL]
        wps = c_proj_w[cs, :]
        rr = bv.astype(np.float64) @ wps.astype(np.float64)
        if g == 0:
            rr = rr + c_proj_b
        in_maps.append(
            {
                "hs": np.ascontiguousarray(hidden_states[b]),
                "wqk": np.ascontiguousarray(
                    np.concatenate([wq, wk, wvs], axis=1).astype(np.float16)
                ),
                "wp": np.ascontiguousarray(wps.astype(np.float16)),
                "bqk": np.ascontiguousarray(np.concatenate([bq, bk])),
                "_rrow": np.ascontiguousarray(rr.astype(np.float32)),
            }
        )
    return in_maps


def kernel(
    hidden_states,
    c_attn_w,
    c_attn_b,
    c_proj_w,
    c_proj_b,
    causal_mask=None,
    **_unused,
):
    from concourse.bass_utils import run_bass_kernel_spmd

    nc = build_kernel()
    in_maps = make_in_maps(
        hidden_states, c_attn_w, c_attn_b, c_proj_w, c_proj_b
    )
    rrows = [m.pop("_rrow") for m in in_maps]
    res = run_bass_kernel_spmd(nc, in_maps, list(range(N_CORES)))
    out = np.zeros((B, S, D), dtype=np.float32)
    for c in range(N_CORES):
        out[c // GROUPS] += res.results[c]["outp"].astype(np.float32)
        out[c // GROUPS] += rrows[c]
    return out


# revision 29
# speedup vs baseline: 1.2854x; 1.2854x over previous
"""GPT-2 style attention block (B=2, S=2048, D=1024, H=16) on 8 TRN2 NeuronCores.

Sharding: tensor-parallel over heads + data-parallel over batch.
Cores 0-3 handle batch 0, cores 4-7 handle batch 1; each core owns 4 of the
16 heads (its 256-column slice of the qkv projection and the matching
256-row slice of c_proj_w). Each core produces a partial output
[S, D] (stored fp16) = ctx_heads @ c_proj_rows; the 4 partials per batch
are summed on the host.

v2 design (fp16 dataflow, measured v1 trace drove these choices):
  1. hs fp32 DMA -> ACT cast to fp16 -> PE transposes (fp16 1-pass, batched
     4 per PSUM bank) -> DVE cast -> hsT fp16.
  2. QKV projections all-fp16 operands (fp32 PSUM accum): 1024-col moving
     chunks; bias applied via ACT Identity drain (fp16 out) -> qkT.
  3. Attention per (qb, hp, kt) with exact causal trimming: diagonal tiles
     stream only the valid 512-128j query columns; a single [128,128]
     upper-triangular fp16 mask handles the in-tile triangle via DVE mul.
     exp on ACT (both heads of the pair in one op, fp32 psum -> fp16 es).
     AV accumulates ctx_aug^T[65, q] in PSUM (ones-column of V_aug gives
     the softmax denominator in row 64).
  4. Normalize: reciprocal_approx_fast on the denominator rows (PSUM ->
     SBUF), gpsimd partition_broadcast, DVE mul -> ctxT fp16.
  5. Output projection fp16 (1024-col moving), DVE PSUM->fp16 copy, DMA out.
PSUM: tp pool (2x1 bank, closed early) / big pool 2x[128,1024] (projections,
scores, outproj) / cx pool 2x[65,2,512]; 8 banks total.

The bias rows (c_attn_b v-slice folded through c_proj_w, plus c_proj_b)
are added on the host during unsharding (exactly zero for the reference
setup_inputs). The causal_mask input is the deterministic tril mask from
setup_inputs(); causality is implemented analytically on device.
"""

import numpy as np

B, S, D, H = 2, 2048, 1024, 16
HD = D // H  # 64
N_CORES = 8
HPC = 4  # heads per core
GROUPS = 4  # cores per batch
HSL = HPC * HD  # 256: per-core head-column width

NDT = D // 128  # 8 contraction tiles
NRT = S // 128  # 16 row tiles
NQB = S // 512  # 4 query blocks

_nc_cache = {}


def _build():
    import concourse.bacc as bacc
    import concourse.mybir as mybir
    import concourse.tile as tile
    from concourse.masks import make_identity, make_upper_triangular

    f32 = mybir.dt.float32
    f16 = mybir.dt.float16

    nc = bacc.Bacc("TRN2", debug=False, num_devices=N_CORES)

    hs = nc.dram_tensor("hs", [S, D], f32, kind="ExternalInput")
    wqk = nc.dram_tensor("wqk", [D, 2 * HSL], f16, kind="ExternalInput")
    wv = nc.dram_tensor("wv", [D, HSL], f16, kind="ExternalInput")
    wp = nc.dram_tensor("wp", [HSL, D], f16, kind="ExternalInput")
    bqk = nc.dram_tensor("bqk", [2 * HSL], f32, kind="ExternalInput")
    outp = nc.dram_tensor("outp", [S, D], f16, kind="ExternalOutput")

    with tile.TileContext(nc) as tc:
        with (
            tc.tile_pool(name="persist", bufs=1) as persist,
            tc.tile_pool(name="hs_in", bufs=3) as hs_pool,
            tc.tile_pool(name="hs16", bufs=3) as h16_pool,
            tc.tile_pool(name="es", bufs=6) as es_pool,
            tc.tile_pool(name="rb", bufs=2) as rb_pool,
            tc.tile_pool(name="ob", bufs=3) as ob_pool,
            tc.tile_pool(name="big", bufs=2, space="PSUM") as big_pool,
        ):
            # ---- persistent SBUF ----
            # rt-major so the PSUM->SBUF cast after each row-tile's
            # transposes writes a contiguous 1024-element span
            hsT = persist.tile([128, NRT, NDT, 128], f16)
            qkT = persist.tile([128, 4, S], f16)  # [Q(2 ct) | K(2 ct)]
            vv = persist.tile([128, NRT, HPC * (HD + 1)], f16)  # V aug
            wqk_sb = persist.tile([128, NDT, 2 * HSL], f16)
            wv_sb = persist.tile([128, NDT, HSL], f16)
            wp_sb = persist.tile([128, 2, D], f16)
            bqk_sb = persist.tile([128, 4], f32)
            ident = persist.tile([128, 128], f16)
            dmask = persist.tile([128, 128], f16)  # 1 where q(col) >= k(row)
            ctxT = persist.tile([128, 2, S], f16)

            make_identity(nc, ident)
            make_upper_triangular(nc, dmask, val=1.0, diag=True)
            # ones columns of V_aug (data cols overwritten by vproj)
            nc.gpsimd.memset(vv, 1.0)

            # ---- phases 1-3, software pipelined per 1024-col s-block:
            # DMA+cast+transpose row tiles, V projection per row tile,
            # then the QK^T projection for the completed s-block.
            tp_ctx = tc.tile_pool(name="tp", bufs=2, space="PSUM")
            tp_pool = tp_ctx.__enter__()
            for ntb in range(S // 1024):
                for rt in range(8 * ntb, 8 * ntb + 8):
                    h_in = hs_pool.tile([128, D], f32)
                    h16 = h16_pool.tile([128, D], f16)
                    # rt0: split DMA+cast in halves so the first transposes
                    # start as early as possible after queue spin-up
                    nh = 2 if rt == 0 else 1
                    hw_ = D // nh
                    for ih in range(nh):
                        cs = slice(ih * hw_, (ih + 1) * hw_)
                        nc.sync.dma_start(
                            out=h_in[:, cs],
                            in_=hs[rt * 128 : (rt + 1) * 128, cs],
                        )
                        nc.scalar.copy(h16[:, cs], h_in[:, cs])
                    pt = tp_pool.tile([128, 1024], f16, tag="tp")
                    for dt in range(NDT):
                        nc.tensor.transpose(
                            pt[:, dt * 128 : (dt + 1) * 128],
                            h16[:, dt * 128 : (dt + 1) * 128],
                            ident,
                        )
                    nc.vector.tensor_copy(
                        hsT[:, rt, :, :],
                        pt.rearrange("p (t c) -> p t c", c=128),
                    )
                    if rt == 0:
                        # weight DMAs after the first hs chunk is queued
                        nc.sync.dma_start(
                            out=wqk_sb,
                            in_=wqk.rearrange("(t p) n -> p t n", p=128),
                        )
                        nc.sync.dma_start(
                            out=wv_sb,
                            in_=wv.rearrange("(t p) n -> p t n", p=128),
                        )
                        nc.sync.dma_start(
                            out=wp_sb,
                            in_=wp.rearrange("(t p) n -> p t n", p=128),
                        )
                        nc.sync.dma_start(
                            out=bqk_sb, in_=bqk.rearrange("(t p) -> p t", p=128)
                        )
                    # V projection for this row tile (hsT stationary)
                    pvt = big_pool.tile([128, 1024], f32, tag="pj")
                    pv = pvt[:, :HSL]
                    for dt in range(NDT):
                        nc.tensor.matmul(
                            pv,
                            hsT[:, rt, dt, :],
                            wv_sb[:, dt, :],
                            start=(dt == 0),
                            stop=(dt == NDT - 1),
                        )
                    vtgt = vv[:, rt, :].rearrange("p (h c) -> p h c", c=HD + 1)
                    nc.vector.tensor_copy(
                        vtgt[:, :, 0:HD],
                        pv.rearrange("p (h c) -> p h c", c=HD),
                    )
                # QK^T projection for this s-block (W stationary)
                for ct in range(4):
                    pj = big_pool.tile([128, 1024], f32, tag="pj")
                    for dt in range(NDT):
                        for half in range(2):
                            nc.tensor.matmul(
                                pj[:, half * 512 : (half + 1) * 512],
                                wqk_sb[:, dt, ct * 128 : (ct + 1) * 128],
                                hsT[
                                    :,
                                    8 * ntb + 4 * half : 8 * ntb + 4 * half + 4,
                                    dt,
                                    :,
                                ],
                                start=(dt == 0),
                                stop=(dt == NDT - 1),
                            )
                    nc.scalar.activation(
                        qkT[:, ct, ntb * 1024 : (ntb + 1) * 1024],
                        pj,
                        mybir.ActivationFunctionType.Identity,
                        bias=bqk_sb[:, ct : ct + 1],
                    )
            tp_ctx.__exit__(None, None, None)

            # ---- phase 4: attention + output projection ----
            # Query blocks largest-first so each block's normalize+outproj
            # latency hides under the next block's attention; each qb's
            # outproj is emitted one block late (PE queue is FIFO, so this
            # keeps ready attention matmuls ahead of dependent outproj ones).
            def emit_outproj(qb):
                for mt in range(4 * qb, 4 * qb + 4):
                    po = big_pool.tile([128, 1024], f32, tag="pj")
                    for ht in range(2):
                        for half in range(2):
                            nc.tensor.matmul(
                                po[:, half * 512 : (half + 1) * 512],
                                ctxT[:, ht, mt * 128 : (mt + 1) * 128],
                                wp_sb[:, ht, half * 512 : (half + 1) * 512],
                                start=(ht == 0),
                                stop=(ht == 1),
                            )
                    ob = ob_pool.tile([128, 1024], f16)
                    nc.scalar.copy(ob, po)
                    nc.sync.dma_start(
                        out=outp[mt * 128 : (mt + 1) * 128, :], in_=ob
                    )

            cx_ctx = tc.tile_pool(name="cx", bufs=2, space="PSUM")
            cx_pool = cx_ctx.__enter__()
            prev_qb = None
            for qb in (0, 3, 2, 1):
                for hp in range(2):
                    cx = cx_pool.tile([65, 2, 512], f32, tag="cx")
                    kmax = 4 * (qb + 1)
                    for kt in range(kmax):
                        j = kt - 4 * qb
                        w = 512 if j < 0 else 512 - 128 * j
                        qo = 512 - w
                        scp = big_pool.tile([128, 1024], f32, tag="pj")
                        for hh in range(2):
                            nc.tensor.matmul(
                                scp[:, 512 * hh + qo : 512 * (hh + 1)],
                                qkT[
                                    hh * 64 : (hh + 1) * 64,
                                    2 + hp,
                                    kt * 128 : (kt + 1) * 128,
                                ],
                                qkT[
                                    hh * 64 : (hh + 1) * 64,
                                    hp,
                                    qb * 512 + qo : (qb + 1) * 512,
                                ],
                                start=True,
                                stop=True,
                                tile_position=(hh * 64, 0),
                            )
                        es = es_pool.tile([128, 1024], f16, tag="es")
                        scp3 = scp.rearrange("p (h c) -> p h c", c=512)
                        es3 = es.rearrange("p (h c) -> p h c", c=512)
                        nc.scalar.activation(
                            es3[:, :, qo:512],
                            scp3[:, :, qo:512],
                            mybir.ActivationFunctionType.Exp,
                            scale=float(1.0 / np.sqrt(HD)),
                        )
                        if j >= 0:
                            for hh in range(2):
                                nc.vector.tensor_mul(
                                    es[:, 512 * hh + qo : 512 * hh + qo + 128],
                                    es[:, 512 * hh + qo : 512 * hh + qo + 128],
                                    dmask,
                                )
                        for hh in range(2):
                            h = 2 * hp + hh
                            nc.tensor.matmul(
                                cx[:, hh, qo:512],
                                vv[:, kt, h * (HD + 1) : (h + 1) * (HD + 1)],
                                es[:, 512 * hh + qo : 512 * (hh + 1)],
                                start=(kt == 0),
                                stop=(kt == kmax - 1),
                            )
                    # normalize: row 64 holds the softmax denominator
                    # fp16 denom copy (SBUF) -> fp16 reciprocals; both recips
                    # issued first so hh1's recip overlaps hh0's broadcast
                    den = rb_pool.tile([1, 1024], f16, tag="den")
                    den3 = den.rearrange("p (h c) -> p h c", c=512)
                    nc.vector.tensor_copy(den3, cx[64:65, :, :])
                    rec = rb_pool.tile([1, 1024], f16, tag="rec")
                    rec3 = rec.rearrange("p (h c) -> p h c", c=512)
                    rbt = rb_pool.tile([64, 1024], f16, tag="rbt")
                    rbt3 = rbt.rearrange("p (h c) -> p h c", c=512)
                    for hh in range(2):
                        with nc.allow_low_precision("softmax denom recip"):
                            nc.vector.reciprocal(
                                rec3[:, hh, :], den3[:, hh, :]
                            )
                    for hh in range(2):
                        nc.gpsimd.partition_broadcast(
                            rbt3[:, hh, :], rec3[:, hh, :]
                        )
                        nc.vector.tensor_mul(
                            ctxT[
                                hh * 64 : hh * 64 + 64,
                                hp,
                                qb * 512 : (qb + 1) * 512,
                            ],
                            cx[0:64, hh, :],
                            rbt3[:, hh, :],
                        )
                # emit the PREVIOUS block's outproj now: its normalize has
                # had this block's attention to complete
                if prev_qb is not None:
                    emit_outproj(prev_qb)
                prev_qb = qb
            emit_outproj(prev_qb)
            cx_ctx.__exit__(None, None, None)

    nc.compile()
    return nc


def build_kernel(matmul_dtype=None, av_dtype=None):
    # single fp16 variant; dtype args accepted for harness compat
    if "k" not in _nc_cache:
        _nc_cache["k"] = _build()
    return _nc_cache["k"]


def make_in_maps(
    hidden_states, c_attn_w, c_attn_b, c_proj_w, c_proj_b,
    matmul_dtype=None, av_dtype=None,
):
    hidden_states = np.asarray(hidden_states, dtype=np.float32)
    c_attn_w = np.asarray(c_attn_w, dtype=np.float32)
    c_attn_b = np.asarray(c_attn_b, dtype=np.float32)
    c_proj_w = np.asarray(c_proj_w, dtype=np.float32)
    c_proj_b = np.asarray(c_proj_b, dtype=np.float32)

    in_maps = []
    for c in range(N_CORES):
        b, g = divmod(c, GROUPS)
        cs = slice(g * HSL, (g + 1) * HSL)
        wq = c_attn_w[:, g * HSL : (g + 1) * HSL]
        wk = c_attn_w[:, D + g * HSL : D + (g + 1) * HSL]
        wvs = c_attn_w[:, 2 * D + g * HSL : 2 * D + (g + 1) * HSL]
        bq = c_attn_b[g * HSL : (g + 1) * HSL]
        bk = c_attn_b[D + g * HSL : D + (g + 1) * HSL]
        bv = c_attn_b[2 * D + g * HSL : 2 * D + (g + 1) * HSL]
        wps = c_proj_w[cs, :]
        rr = bv.astype(np.float64) @ wps.astype(np.float64)
        if g == 0:
            rr = rr + c_proj_b
        in_maps.append(
            {
                "hs": np.ascontiguousarray(hidden_states[b]),
                "wqk": np.ascontiguousarray(
                    np.concatenate([wq, wk], axis=1).astype(np.float16)
                ),
                "wv": np.ascontiguousarray(wvs.astype(np.float16)),
                "wp": np.ascontiguousarray(wps.astype(np.float16)),
                "bqk": np.ascontiguousarray(np.concatenate([bq, bk])),
                "_rrow": np.ascontiguousarray(rr.astype(np.float32)),
            }
        )
    return in_maps


def kernel(
    hidden_states,
    c_attn_w,
    c_attn_b,
    c_proj_w,
    c_proj_b,
    causal_mask=None,
    **_unused,
):
    from concourse.bass_utils import run_bass_kernel_spmd

    nc = build_kernel()
    in_maps = make_in_maps(
        hidden_states, c_attn_w, c_attn_b, c_proj_w, c_proj_b
    )
    rrows = [m.pop("_rrow") for m in in_maps]
    res = run_bass_kernel_spmd(nc, in_maps, list(range(N_CORES)))
    out = np.zeros((B, S, D), dtype=np.float32)
    for c in range(N_CORES):
        out[c // GROUPS] += res.results[c]["outp"].astype(np.float32)
        out[c // GROUPS] += rrows[c]
    return out
